# revision 1
# baseline (speedup 1.0000x reference)
"""HalfKP input layer (dual GEMV + bias + relu) on 8 Trainium2 NeuronCores.

out[512] = concat(relu(W_my @ x[:41024] + b_my), relu(W_opp @ x[41024:] + b_opp))

Sharding: 512 output rows split 64 rows/core (output-feature parallel; cores
0-3 handle W_my, 4-7 handle W_opp).  Per core the [64, 41024] shard is
host-repacked into [128, 16*1282]: partition p = rr*32 + b holds row
(t*4 + rr)'s k-block b (kb=1282) at free offset t*1282.  The device streams W
in 1.31 MB DMAs (10.3 KB contiguous runs per partition), runs 16 fused
multiply+reduce custom-DVE ops (TENSOR_TENSOR_REDUCE against a [128, 1282]
x-block tile, bias seeded via s0), contracts the 32 k-block partials per row
with one tiny PE matmul, applies relu on DVE, and writes a [4, 16] result per
core.  Memory-roofline bound: ~10.5 MB HBM reads per core (~400 GB/s/core
measured stream rate).
"""

import numpy as np

K = 41024          # features per side
B = 32             # k-blocks per row
KB = K // B        # 1282 elements per k-block
R = 128 // B       # 4 rows processed per DVE op
T = 64 // R        # 16 DVE ops (row groups) per core
CHUNK = 2          # TTR ops per W DMA (10.3KB contiguous runs per partition)
XCOLS = KB + R + T  # xq | mask[4] | seed[16]
N_CORES = 8
ROWS_PER_CORE = 64

_compiled = None


def _build_nc():
    import concourse.bacc as bacc
    import concourse.mybir as mybir
    import concourse.tile as tile
    from concourse.dve_ops import TENSOR_TENSOR_REDUCE

    F32 = mybir.dt.float32

    nc = bacc.Bacc("TRN2", target_bir_lowering=False, debug=False)

    wt_d = nc.dram_tensor("wt", [128, T * KB], F32, kind="ExternalInput")
    xqp_d = nc.dram_tensor("xqp", [128, XCOLS], F32, kind="ExternalInput")
    out_d = nc.dram_tensor("out", [R, T], F32, kind="ExternalOutput")

    def ttr(w_ap, xq_ap, seed_ap, acc_ap, prod_ap):
        # out = in0*in1*s1; accum = s0 + sum(out)  (custom-DVE ucode op)
        nc.vector._custom_dve(
            TENSOR_TENSOR_REDUCE,
            out=prod_ap,
            in0=w_ap,
            in1=xq_ap,
            s0=seed_ap,
            s1=1.0,
            accum_out=acc_ap,
        )

    n_chunks = T // CHUNK
    with tile.TileContext(nc) as tc:
        with (
            tc.tile_pool(name="const", bufs=1) as constp,
            tc.tile_pool(name="w", bufs=n_chunks + 1) as wp,
            tc.tile_pool(name="scratch", bufs=1) as sp,
            tc.tile_pool(name="ps", bufs=1, space="PSUM") as psp,
        ):
            # xqp rides the scalar (ACT) HWDGE ring so the W stream on the
            # sync ring starts immediately and both make progress in parallel
            xqp = constp.tile([128, XCOLS], F32, tag="xqp")
            nc.scalar.dma_start(xqp[:], xqp_d[:])
            xq = xqp[:, 0:KB]
            mask = xqp[:, KB : KB + R]
            seed = xqp[:, KB + R : KB + R + T]

            acc = constp.tile([128, T], F32, tag="acc")
            prod = sp.tile([128, KB], F32, tag="prod")

            for c in range(n_chunks):
                w_sb = wp.tile([128, CHUNK * KB], F32, tag="w")
                nc.sync.dma_start(
                    w_sb[:], wt_d[:, c * CHUNK * KB : (c + 1) * CHUNK * KB]
                )
                for j in range(CHUNK):
                    t = c * CHUNK + j
                    ttr(
                        w_sb[:, j * KB : (j + 1) * KB],
                        xq,
                        seed[:, t : t + 1],
                        acc[:, t : t + 1],
                        prod[:],
                    )

            ps = psp.tile([R, T], F32, tag="ps")
            nc.tensor.matmul(ps[:], lhsT=mask, rhs=acc[:], start=True, stop=True)
            out_sb = sp.tile([R, T], F32, tag="out")
            nc.vector.tensor_scalar_max(out_sb[:], ps[:], 0.0)
            nc.sync.dma_start(out_d[:], out_sb[:])

    nc.compile()
    return nc


def _get_nc():
    global _compiled
    if _compiled is None:
        _compiled = _build_nc()
    return _compiled


def make_in_maps(input, W_my, b_my, W_opp, b_opp):
    """Host-side sharding: per-core input dicts."""
    x = np.ascontiguousarray(input, dtype=np.float32)
    Wcat = np.concatenate(
        [np.asarray(W_my, np.float32), np.asarray(W_opp, np.float32)], axis=0
    )
    bcat = np.concatenate(
        [np.asarray(b_my, np.float32), np.asarray(b_opp, np.float32)]
    )

    mask = (np.arange(128)[:, None] // B == np.arange(R)[None, :]).astype(np.float32)

    in_maps = []
    for c in range(N_CORES):
        Wsh = Wcat[c * ROWS_PER_CORE : (c + 1) * ROWS_PER_CORE]  # [64, K]
        xs = x[:K] if c < 4 else x[K:]
        # wt[p = rr*B + b, t*KB + j] = Wsh[t*R + rr, b*KB + j]
        wt = np.ascontiguousarray(
            Wsh.reshape(T, R, B, KB).transpose(1, 2, 0, 3).reshape(128, T * KB)
        )
        bsh = bcat[c * ROWS_PER_CORE : (c + 1) * ROWS_PER_CORE]
        seed = np.zeros((128, T), np.float32)
        # partition rr*B (b == 0) seeds the bias for row t*R + rr
        seed[np.arange(R) * B, :] = bsh.reshape(T, R).T
        xqp = np.empty((128, XCOLS), np.float32)
        xqp[:, 0:KB] = np.tile(xs.reshape(B, KB), (R, 1))
        xqp[:, KB : KB + R] = mask
        xqp[:, KB + R :] = seed
        in_maps.append({"wt": wt, "xqp": xqp})
    return in_maps


def gather_output(results):
    """results: list of per-core dicts with 'out' [R, T] -> full [512]."""
    outs = []
    for c in range(N_CORES):
        o = np.asarray(results[c]["out"], np.float32)  # [R, T]
        outs.append(o.T.ravel())  # row r = t*R + rr
    return np.concatenate(outs)


def run_on_hw(in_maps, trace=False, **kwargs):
    from concourse.bass_utils import run_bass_kernel_spmd

    nc = _get_nc()
    return run_bass_kernel_spmd(
        nc, in_maps, core_ids=list(range(N_CORES)), trace=trace, **kwargs
    )


def kernel(input, W_my, b_my, W_opp, b_opp):
    in_maps = make_in_maps(input, W_my, b_my, W_opp, b_opp)
    res = run_on_hw(in_maps)
    return gather_output(res.results)



# revision 2
# speedup vs baseline: 1.5044x; 1.5044x over previous
"""HalfKP input layer (dual GEMV + bias + relu) on 8 Trainium2 NeuronCores.

out[512] = concat(relu(W_my @ x[:41024] + b_my), relu(W_opp @ x[41024:] + b_opp))

Memory-roofline kernel: the 84 MB f32 weight stream is the whole cost, so
weights ship as fp16 (2.8e-4 end-to-end rel err) and each core streams a
5.3 MB shard at the ~380 GB/s HBM-per-core limit.

Sharding: 4 row-groups x 2 k-halves.  Core c = 2*G + h holds output rows
[128*G, 128*G+128) and k-slice h of its side's 41024 inputs (20512 each,
zero-padded to 161 blocks of 128).  Compute rides the PE in GEMV mode:
for each k-block g, matmul(stationary = x_g [128,1], moving = W_g^T
[128,128]) accumulates partials into a PSUM [1,128] tile — the PE consumes
weights at ~300 Gelem/s, faster than DMA delivers, so the stream is the
only bottleneck and the DVE never touches the hot path.  Cores return raw
f32 partials; the host sums k-half pairs, adds bias, applies relu.
"""

import numpy as np

K = 41024            # features per side
KH = K // 2          # 20512 per k-half shard
NB = 161             # 128-element k-blocks per core (20608, zero-padded)
KP = NB * 128
N_CORES = 8
ROWS = 128           # output rows per core
# W chunk sizes (blocks): small head for an early PE start, small tail so
# the last chunk's matmuls + output DMA are off the critical path quickly.
CHUNKS = [8, 25, 32, 32, 32, 24, 8]
assert sum(CHUNKS) == NB

_compiled = None


def _build_nc():
    import concourse.bacc as bacc
    import concourse.mybir as mybir
    import concourse.tile as tile

    F32 = mybir.dt.float32
    F16 = mybir.dt.float16

    nc = bacc.Bacc("TRN2", target_bir_lowering=False, debug=False)

    wt_d = nc.dram_tensor("wt", [128, KP], F16, kind="ExternalInput")
    xq_d = nc.dram_tensor("xq", [128, NB], F16, kind="ExternalInput")
    out_d = nc.dram_tensor("out", [1, ROWS], F32, kind="ExternalOutput")

    with tile.TileContext(nc) as tc:
        with (
            tc.tile_pool(name="const", bufs=1) as constp,
            tc.tile_pool(name="w", bufs=len(CHUNKS) + 1) as wp,
            tc.tile_pool(name="ps", bufs=1, space="PSUM") as psp,
        ):
            # x rides the scalar (ACT) HWDGE ring so the W stream on the
            # sync ring starts immediately and both progress in parallel
            xq = constp.tile([128, NB], F16, tag="xq")
            nc.scalar.dma_start(xq[:], xq_d[:])

            ps = psp.tile([1, ROWS], F32, tag="ps")

            g = 0
            for c, nblk in enumerate(CHUNKS):
                w_sb = wp.tile([128, nblk * 128], F16, tag="w")
                nc.sync.dma_start(
                    w_sb[:], wt_d[:, g * 128 : (g + nblk) * 128]
                )
                for j in range(nblk):
                    nc.tensor.matmul(
                        ps[:],
                        lhsT=xq[:, g + j : g + j + 1],
                        rhs=w_sb[:, j * 128 : (j + 1) * 128],
                        start=(g + j == 0),
                        stop=(g + j == NB - 1),
                    )
                g += nblk

            out_sb = constp.tile([1, ROWS], F32, tag="out")
            nc.scalar.copy(out_sb[:], ps[:])
            nc.scalar.dma_start(out_d[:], out_sb[:])

    nc.compile()
    return nc


def _get_nc():
    global _compiled
    if _compiled is None:
        _compiled = _build_nc()
    return _compiled


def make_in_maps(input, W_my, b_my, W_opp, b_opp):
    """Host-side sharding: per-core input dicts."""
    x = np.asarray(input, np.float32)
    Wcat = np.concatenate(
        [np.asarray(W_my, np.float32), np.asarray(W_opp, np.float32)], axis=0
    )

    in_maps = []
    for core in range(N_CORES):
        G, h = divmod(core, 2)
        rows = Wcat[G * 128 : (G + 1) * 128]          # [128, K]
        xs = x[:K] if G < 2 else x[K:]
        ksl = slice(h * KH, (h + 1) * KH)

        Wp = np.zeros((128, KP), np.float16)
        Wp[:, :KH] = rows[:, ksl]
        # wt[p, g*128 + j] = Wp[j, g*128 + p]  (k-in-block partition-major)
        wt = np.ascontiguousarray(
            Wp.reshape(128, NB, 128).transpose(2, 1, 0).reshape(128, KP)
        )
        xp = np.zeros(KP, np.float16)
        xp[:KH] = xs[ksl]
        xq = np.ascontiguousarray(xp.reshape(NB, 128).T)  # [128, NB]
        in_maps.append({"wt": wt, "xq": xq})
    return in_maps


def gather_output(results, b_my, b_opp):
    """results: per-core {'out': [1, 128]} raw partials -> full [512]."""
    bcat = np.concatenate(
        [np.asarray(b_my, np.float32), np.asarray(b_opp, np.float32)]
    )
    outs = []
    for G in range(4):
        p = (
            np.asarray(results[2 * G]["out"], np.float32)[0]
            + np.asarray(results[2 * G + 1]["out"], np.float32)[0]
        )
        outs.append(np.maximum(p + bcat[G * 128 : (G + 1) * 128], 0.0))
    return np.concatenate(outs)


def run_on_hw(in_maps, trace=False, **kwargs):
    from concourse.bass_utils import run_bass_kernel_spmd

    nc = _get_nc()
    return run_bass_kernel_spmd(
        nc, in_maps, core_ids=list(range(N_CORES)), trace=trace, **kwargs
    )


def kernel(input, W_my, b_my, W_opp, b_opp):
    in_maps = make_in_maps(input, W_my, b_my, W_opp, b_opp)
    res = run_on_hw(in_maps)
    return gather_output(res.results, b_my, b_opp)
